# revision 34
# baseline (speedup 1.0000x reference)
"""Trainium2 Bass kernel for nn_ActivationFilter:
y = bicubic_down2x( leaky_relu( bicubic_up2x(x) ) ), x: (8, 128, 128, 128) f32 NHWC.

Since jax.image.resize is a separable linear map, per (batch, channel):
    y = D @ leaky_relu(U @ X @ U^T) @ D^T
with U (256x128) the bicubic 2x-upsample matrix and D (128x256) the
antialiased bicubic downsample matrix.

Sharding: batch-per-core (8 batches over 8 NeuronCores), no collectives.

v2 (all-bf16 + quad restructure):
  - Everything on the matmul path is bf16 (x pre-cast host-side): every
    LDWEIGHTS gets Fast Weight Load and input DMA traffic halves.
  - Channel quads (4 ch) per loop iteration; channel pairs share PSUM
    tiles within a quad.
  - PSUM budget (8 banks): ps1 [128,512]x1, ps2 [128,1024]x2,
    ps3 [128,512]x2, ps4-quad [128,512]x1.
  - Evac split: ACT = Prelu (zs) + y-quad copy; DVE = z1 cast + y3 cast
    (y3 alternates to ACT every other pair for balance).
  - Output staged per-quad in SBUF then DMA'd (32 output DMAs).

Per-core phases (per channel c, all on TensorE, no transposes):
  Ph1: z1[w, (k,h2)]   = sum_h  x[h,w,c]*Ut[h,h2]     (lhsT = X_c)
  Ph2: zs[p, (k,b,w2)] = sum_w  z1[w,h2]*Ut[w,w2]     (lhsT = z1 quarter)
       leaky_relu fused into the PSUM evacuation (ScalarE Prelu)
  Ph3: y3[p, (k,a,h3)] = sum_h2 zs[h2,w2]*Dt[h2,h3]   (lhsT = zs slice)
  Ph4: y[w3, (pr,k,h3)]= sum_w2 Dw[w2,w3]^T y3        (lhsT = Dw const)
"""

import sys
import os

if "/opt/trn_rl_repo" not in sys.path:
    sys.path.insert(0, "/opt/trn_rl_repo")

import numpy as np

H = W = C = 128
H2 = W2 = 256
NEG_SLOPE = 0.01


def _keys_cubic(t):
    t = np.abs(t)
    return np.where(
        t <= 1,
        (1.5 * t - 2.5) * t * t + 1,
        np.where(t < 2, ((-0.5 * t + 2.5) * t - 4) * t + 2, 0.0),
    )


def _resize_mat(n_in, n_out, antialias=True):
    """Replicates jax.image.resize(method='bicubic', antialias=True) weights.
    Returns (n_out, n_in) f32 so that y = Wmat @ x along the resized dim."""
    scale = n_out / n_in
    inv_scale = 1.0 / scale
    kernel_scale = max(inv_scale, 1.0) if antialias else 1.0
    sample_f = (np.arange(n_out, dtype=np.float64) + 0.5) * inv_scale - 0.5
    x = (
        np.abs(sample_f[:, None] - np.arange(n_in, dtype=np.float64)[None, :])
        / kernel_scale
    )
    w = _keys_cubic(x)
    total = w.sum(axis=1, keepdims=True)
    w = np.where(np.abs(total) > 1000 * np.finfo(np.float32).eps, w / total, 0)
    w = np.where(((sample_f >= -0.5) & (sample_f <= n_in - 0.5))[:, None], w, 0)
    return w.astype(np.float32)


_BUILD_CACHE = {}


def _build_module():
    """Build + compile the single-core Bass program (same program on all cores)."""
    if "nc" in _BUILD_CACHE:
        return _BUILD_CACHE["nc"]

    import concourse.bacc as bacc
    import concourse.mybir as mybir
    import concourse.tile as tile

    dt = mybir.dt

    nc = bacc.Bacc("TRN2", target_bir_lowering=False, debug=False)

    NQ = 16
    CQ = C // NQ  # 8 channels per input/output quarter
    xins = [
        nc.dram_tensor(f"xin{q}", (H, W * CQ), dt.bfloat16, kind="ExternalInput").ap()
        for q in range(NQ)
    ]
    wh = nc.dram_tensor("wh", (H, H2), dt.bfloat16, kind="ExternalInput").ap()
    dh = nc.dram_tensor("dh", (2, 128, 128), dt.bfloat16, kind="ExternalInput").ap()
    youts = [
        nc.dram_tensor(f"yout{q}", (W, H * CQ), dt.float32, kind="ExternalOutput").ap()
        for q in range(NQ)
    ]

    AFT = mybir.ActivationFunctionType

    with tile.TileContext(nc) as tc:
        with (
            tc.tile_pool(name="big", bufs=1) as bigpool,
            tc.tile_pool(name="const", bufs=1) as cpool,
            tc.tile_pool(name="work", bufs=1) as wpool,
            tc.tile_pool(name="ps1", bufs=2, space="PSUM") as ps1,
            tc.tile_pool(name="ps2", bufs=2, space="PSUM") as ps2,
            tc.tile_pool(name="ps3", bufs=1, space="PSUM") as ps3,
            tc.tile_pool(name="ps4", bufs=1, space="PSUM") as ps4,
        ):
            x_sbs = [
                bigpool.tile([H, W * CQ], dt.bfloat16, tag=f"x{q}", name=f"x_sb{q}")
                for q in range(NQ)
            ]
            wh_sb = cpool.tile([H, H2], dt.bfloat16)
            dh_sb = cpool.tile([128, 256], dt.bfloat16)
            dw_sb = cpool.tile([128, 256], dt.bfloat16)

            x_rs = [t[:].rearrange("p (c w) -> p c w", c=CQ) for t in x_sbs]
            nc.sync.dma_start(out=x_sbs[0][:], in_=xins[0][:])
            nc.sync.dma_start(out=wh_sb[:], in_=wh[:])
            nc.sync.dma_start(out=dh_sb[:, 0:128], in_=dh[0])
            nc.sync.dma_start(out=dh_sb[:, 128:256], in_=dh[1])
            nc.sync.dma_start(out=dw_sb[:, 0:128], in_=dh[0])
            nc.sync.dma_start(out=dw_sb[:, 128:256], in_=dh[1])
            for q in range(1, NQ):
                nc.sync.dma_start(out=x_sbs[q][:], in_=xins[q][:])

            sim_relu = os.environ.get("AF_SIM_RELU", "0") == "1"
            NP = C // 2  # 64 channel pairs
            z1s = [None] * NP
            zss = [None] * NP
            y3s = [None] * NP
            p4ts = {}

            # Full 4-deep software pipeline: at iteration i, phase j runs
            # for pair i-j, so every cross-engine dependency (PSUM evac ->
            # next matmul phase) is a full pipeline cycle old and no engine
            # ever waits on another within the cycle.
            for i in range(NP + 4):
                if 1 <= i < NP + 1:
                    # ---- Ph2: zs (w2half-b, (b, k, h2)); lhsT = Uw halves (const) ----
                    # Emitted FIRST within the iteration so its Prelu evac
                    # lands a full cycle before Ph3 consumes it.
                    j = i - 1
                    z1 = z1s[j][:]
                    p2t = ps2.tile([128, 1024], dt.float32, tag="p2", bufs=2)
                    for b in range(2):
                        nc.tensor.matmul(
                            p2t[:, b * 512 : b * 512 + 512],
                            lhsT=wh_sb[:, b * 128 : b * 128 + 128],
                            rhs=z1,
                            start=True,
                            stop=True,
                        )
                    z1s[j] = None
                    # leaky_relu fused into the PSUM evacuation (ScalarE Prelu)
                    zs = wpool.tile([128, 1024], dt.bfloat16, tag="zs", bufs=4)
                    if sim_relu:
                        nc.scalar.activation(zs[:], p2t[:], AFT.Relu)
                    else:
                        nc.scalar.activation(zs[:], p2t[:], AFT.Prelu, alpha=NEG_SLOPE)
                    zss[j] = zs

                if i < NP:
                    # ---- Ph1: z1 (w, (k, h2)) ----
                    c0 = 2 * i
                    p1t = ps1.tile([128, 512], dt.float32, tag="p1", bufs=2)
                    for k in range(2):
                        nc.tensor.matmul(
                            p1t[:, k * 256 : k * 256 + 256],
                            lhsT=x_rs[(c0 + k) // CQ][:, (c0 + k) % CQ, :],
                            rhs=wh_sb[:],
                            start=True,
                            stop=True,
                        )
                    z1 = wpool.tile([128, 512], dt.bfloat16, tag="z1", bufs=4)
                    nc.vector.tensor_copy(out=z1[:], in_=p1t[:])
                    z1s[i] = z1

                if 2 <= i < NP + 2:
                    # ---- Ph3: y3 (h2half-hh, (k, hh, w3)); lhsT = zs slices ----
                    j = i - 2
                    zs = zss[j]
                    p3t = ps3.tile([128, 512], dt.float32, tag="p3", bufs=1)
                    for k in range(2):
                        for hh in range(2):
                            o = k * 256 + hh * 128
                            for b in range(2):
                                nc.tensor.matmul(
                                    p3t[:, o : o + 128],
                                    lhsT=zs[
                                        :,
                                        b * 512 + k * 256 + hh * 128 : b * 512
                                        + k * 256
                                        + hh * 128
                                        + 128,
                                    ],
                                    rhs=dw_sb[:, b * 128 : b * 128 + 128],
                                    start=(b == 0),
                                    stop=(b == 1),
                                )
                    zss[j] = None
                    y3 = wpool.tile([128, 512], dt.bfloat16, tag="y3", bufs=4)
                    nc.vector.tensor_copy(out=y3[:], in_=p3t[:])
                    y3s[j] = y3[:]

                if 3 <= i < NP + 3:
                    # ---- Ph4: y (h3, (pr, k, w3)); lhsT = Dh const halves ----
                    j = i - 3
                    qd = j // 2
                    if j % 2 == 0:
                        p4ts[qd] = ps4.tile(
                            [128, 512], dt.float32, tag="p4", bufs=1, name=f"p4t{qd}"
                        )
                    p4_r = p4ts[qd][:].rearrange("p (r k n) -> p r k n", r=2, k=2)
                    y3_r = y3s[j].rearrange("p (k hh n) -> p k hh n", k=2, hh=2)
                    for hh in range(2):
                        nc.tensor.matmul(
                            p4_r[:, j % 2],
                            lhsT=dh_sb[:, hh * 128 : hh * 128 + 128],
                            rhs=y3_r[:, :, hh, :],
                            start=(hh == 0),
                            stop=(hh == 1),
                        )
                    y3s[j] = None
                    if j % 2 == 1:
                        # quad output: PSUM -> SBUF (ACT) -> DRAM
                        yq = wpool.tile([128, 512], dt.float32, tag="yq", bufs=3)
                        nc.scalar.activation(yq[:], p4ts[qd][:], AFT.Copy)
                        qt, off = (4 * qd) // CQ, ((4 * qd) % CQ) * H
                        nc.sync.dma_start(
                            out=youts[qt][:, off : off + 512], in_=yq[:]
                        )

    nc.compile()
    _BUILD_CACHE["nc"] = nc
    return nc


def _input_maps(x):
    U = _resize_mat(H, H2)   # (256, 128) upsample
    D = _resize_mat(H2, H)   # (128, 256) antialiased downsample
    try:
        from ml_dtypes import bfloat16
    except ImportError:
        import jax.numpy as jnp  # fallback
        bfloat16 = jnp.bfloat16

    wh_np = np.ascontiguousarray(U.T).astype(bfloat16)     # (h, h2)
    # dh[b, h2local, h3] = D[h3, b*128 + h2local]
    dh_np = np.ascontiguousarray(D.T.reshape(2, 128, 128)).astype(bfloat16)

    in_maps = []
    for i in range(x.shape[0]):
        xr = x[i].reshape(H, W, C).astype(bfloat16)
        m = {"wh": wh_np, "dh": dh_np}
        for q in range(16):
            m[f"xin{q}"] = np.ascontiguousarray(
                xr[:, :, q * 8 : (q + 1) * 8].transpose(0, 2, 1)
            ).reshape(H, W * 8)
        in_maps.append(m)
    return in_maps


def _unshard(results):
    outs = []
    for r in results:
        qs = [np.asarray(r[f"yout{q}"]).reshape(H, 8, W).transpose(0, 2, 1) for q in range(16)]
        outs.append(np.concatenate(qs, axis=2))     # (h3, w3, c)
    return np.stack(outs, axis=0).astype(np.float32)


def run(x, trace=False):
    """Run on 8 NeuronCores. Returns (y, exec_time_ns or None)."""
    from concourse.bass_utils import run_bass_kernel_spmd

    nc = _build_module()
    in_maps = _input_maps(np.asarray(x, dtype=np.float32))
    core_ids = list(range(len(in_maps)))
    res = run_bass_kernel_spmd(nc, in_maps, core_ids, trace=trace)
    return _unshard(res.results), res.exec_time_ns


def kernel(x):
    y, _ = run(x, trace=False)
    return y


def _run_sim(x_batch):
    """CoreSim single-core numerical check (x_batch: (128,128,128) f32)."""
    import concourse.bass_interp as bass_interp

    nc = _build_module()
    sim = bass_interp.CoreSim(nc, trace=False)
    im = _input_maps(x_batch[None])[0]
    for k, v in im.items():
        sim.tensor(k)[:] = v
    sim.simulate()
    qs = [np.asarray(sim.tensor(f"yout{q}")).reshape(H, 8, W).transpose(0, 2, 1) for q in range(16)]
    return np.concatenate(qs, axis=2)


# revision 39
# speedup vs baseline: 1.0229x; 1.0229x over previous
"""Trainium2 Bass kernel for nn_ActivationFilter:
y = bicubic_down2x( leaky_relu( bicubic_up2x(x) ) ), x: (8, 128, 128, 128) f32 NHWC.

Since jax.image.resize is a separable linear map, per (batch, channel):
    y = D @ leaky_relu(U @ X @ U^T) @ D^T
with U (256x128) the bicubic 2x-upsample matrix and D (128x256) the
antialiased bicubic downsample matrix.

Sharding: batch-per-core (8 batches over 8 NeuronCores), no collectives.

v2 (all-bf16 + quad restructure):
  - Everything on the matmul path is bf16 (x pre-cast host-side): every
    LDWEIGHTS gets Fast Weight Load and input DMA traffic halves.
  - Channel quads (4 ch) per loop iteration; channel pairs share PSUM
    tiles within a quad.
  - PSUM budget (8 banks): ps1 [128,512]x1, ps2 [128,1024]x2,
    ps3 [128,512]x2, ps4-quad [128,512]x1.
  - Evac split: ACT = Prelu (zs) + y-quad copy; DVE = z1 cast + y3 cast
    (y3 alternates to ACT every other pair for balance).
  - Output staged per-quad in SBUF then DMA'd (32 output DMAs).

Per-core phases (per channel c, all on TensorE, no transposes):
  Ph1: z1[w, (k,h2)]   = sum_h  x[h,w,c]*Ut[h,h2]     (lhsT = X_c)
  Ph2: zs[p, (k,b,w2)] = sum_w  z1[w,h2]*Ut[w,w2]     (lhsT = z1 quarter)
       leaky_relu fused into the PSUM evacuation (ScalarE Prelu)
  Ph3: y3[p, (k,a,h3)] = sum_h2 zs[h2,w2]*Dt[h2,h3]   (lhsT = zs slice)
  Ph4: y[w3, (pr,k,h3)]= sum_w2 Dw[w2,w3]^T y3        (lhsT = Dw const)
"""

import sys
import os

if "/opt/trn_rl_repo" not in sys.path:
    sys.path.insert(0, "/opt/trn_rl_repo")

import numpy as np

H = W = C = 128
H2 = W2 = 256
NEG_SLOPE = 0.01


def _keys_cubic(t):
    t = np.abs(t)
    return np.where(
        t <= 1,
        (1.5 * t - 2.5) * t * t + 1,
        np.where(t < 2, ((-0.5 * t + 2.5) * t - 4) * t + 2, 0.0),
    )


def _resize_mat(n_in, n_out, antialias=True):
    """Replicates jax.image.resize(method='bicubic', antialias=True) weights.
    Returns (n_out, n_in) f32 so that y = Wmat @ x along the resized dim."""
    scale = n_out / n_in
    inv_scale = 1.0 / scale
    kernel_scale = max(inv_scale, 1.0) if antialias else 1.0
    sample_f = (np.arange(n_out, dtype=np.float64) + 0.5) * inv_scale - 0.5
    x = (
        np.abs(sample_f[:, None] - np.arange(n_in, dtype=np.float64)[None, :])
        / kernel_scale
    )
    w = _keys_cubic(x)
    total = w.sum(axis=1, keepdims=True)
    w = np.where(np.abs(total) > 1000 * np.finfo(np.float32).eps, w / total, 0)
    w = np.where(((sample_f >= -0.5) & (sample_f <= n_in - 0.5))[:, None], w, 0)
    return w.astype(np.float32)


_BUILD_CACHE = {}


def _build_module():
    """Build + compile the single-core Bass program (same program on all cores)."""
    if "nc" in _BUILD_CACHE:
        return _BUILD_CACHE["nc"]

    import concourse.bacc as bacc
    import concourse.mybir as mybir
    import concourse.tile as tile

    dt = mybir.dt

    nc = bacc.Bacc("TRN2", target_bir_lowering=False, debug=False)

    NQ = 16
    CQ = C // NQ  # 8 channels per input/output quarter
    xins = [
        nc.dram_tensor(f"xin{q}", (H, W * CQ), dt.bfloat16, kind="ExternalInput").ap()
        for q in range(NQ)
    ]
    wh = nc.dram_tensor("wh", (H, H2), dt.bfloat16, kind="ExternalInput").ap()
    dh = nc.dram_tensor("dh", (2, 128, 128), dt.bfloat16, kind="ExternalInput").ap()
    youts = [
        nc.dram_tensor(f"yout{q}", (W, H * CQ), dt.float32, kind="ExternalOutput").ap()
        for q in range(NQ)
    ]

    AFT = mybir.ActivationFunctionType

    with tile.TileContext(nc) as tc:
        with (
            tc.tile_pool(name="big", bufs=1) as bigpool,
            tc.tile_pool(name="const", bufs=1) as cpool,
            tc.tile_pool(name="work", bufs=1) as wpool,
            tc.tile_pool(name="ps1", bufs=2, space="PSUM") as ps1,
            tc.tile_pool(name="ps2", bufs=2, space="PSUM") as ps2,
            tc.tile_pool(name="ps3", bufs=1, space="PSUM") as ps3,
            tc.tile_pool(name="ps4", bufs=1, space="PSUM") as ps4,
        ):
            x_sbs = [
                bigpool.tile([H, W * CQ], dt.bfloat16, tag=f"x{q}", name=f"x_sb{q}")
                for q in range(NQ)
            ]
            wh_sb = cpool.tile([H, H2], dt.bfloat16)
            dh_sb = cpool.tile([128, 256], dt.bfloat16)
            dw_sb = cpool.tile([128, 256], dt.bfloat16)

            x_rs = [t[:].rearrange("p (c w) -> p c w", c=CQ) for t in x_sbs]
            nc.sync.dma_start(out=x_sbs[0][:], in_=xins[0][:])
            nc.sync.dma_start(out=wh_sb[:], in_=wh[:])
            nc.sync.dma_start(out=dh_sb[:, 0:128], in_=dh[0])
            nc.sync.dma_start(out=dh_sb[:, 128:256], in_=dh[1])
            nc.sync.dma_start(out=dw_sb[:, 0:128], in_=dh[0])
            nc.sync.dma_start(out=dw_sb[:, 128:256], in_=dh[1])
            for q in range(1, NQ):
                nc.sync.dma_start(out=x_sbs[q][:], in_=xins[q][:])

            sim_relu = os.environ.get("AF_SIM_RELU", "0") == "1"
            NP = C // 2  # 64 channel pairs
            z1s = [None] * NP
            zss = [None] * NP
            y3s = [None] * NP
            p4ts = {}

            # Full 4-deep software pipeline: at iteration i, phase j runs
            # for pair i-j, so every cross-engine dependency (PSUM evac ->
            # next matmul phase) is a full pipeline cycle old and no engine
            # ever waits on another within the cycle.
            for i in range(NP + 4):
                if 1 <= i < NP + 1:
                    # ---- Ph2: zs (w2half-b, (b, k, h2)); lhsT = Uw halves (const) ----
                    # Emitted FIRST within the iteration so its Prelu evac
                    # lands a full cycle before Ph3 consumes it.
                    j = i - 1
                    z1 = z1s[j][:]
                    p2t = ps2.tile([128, 1024], dt.float32, tag="p2", bufs=2)
                    for b in range(2):
                        nc.tensor.matmul(
                            p2t[:, b * 512 : b * 512 + 512],
                            lhsT=wh_sb[:, b * 128 : b * 128 + 128],
                            rhs=z1,
                            start=True,
                            stop=True,
                        )
                    z1s[j] = None
                    # leaky_relu fused into the PSUM evacuation (ScalarE Prelu)
                    zs = wpool.tile([128, 1024], dt.bfloat16, tag="zs", bufs=4)
                    if sim_relu:
                        nc.scalar.activation(zs[:], p2t[:], AFT.Relu)
                    else:
                        nc.scalar.activation(zs[:], p2t[:], AFT.Prelu, alpha=NEG_SLOPE)
                    zss[j] = zs

                if i < NP:
                    # ---- Ph1: z1 (w, (k, h2)) ----
                    c0 = 2 * i
                    p1t = ps1.tile([128, 512], dt.float32, tag="p1", bufs=2)
                    for k in range(2):
                        nc.tensor.matmul(
                            p1t[:, k * 256 : k * 256 + 256],
                            lhsT=x_rs[(c0 + k) // CQ][:, (c0 + k) % CQ, :],
                            rhs=wh_sb[:],
                            start=True,
                            stop=True,
                        )
                    z1 = wpool.tile([128, 512], dt.bfloat16, tag="z1", bufs=4)
                    nc.vector.tensor_copy(out=z1[:], in_=p1t[:])
                    z1s[i] = z1

                if 2 <= i < NP + 2:
                    # ---- Ph3: y3 (h2half-hh, (k, hh, w3)); lhsT = zs slices ----
                    j = i - 2
                    zs = zss[j]
                    p3t = ps3.tile([128, 512], dt.float32, tag="p3", bufs=1)
                    for k in range(2):
                        for hh in range(2):
                            o = k * 256 + hh * 128
                            for b in range(2):
                                nc.tensor.matmul(
                                    p3t[:, o : o + 128],
                                    lhsT=zs[
                                        :,
                                        b * 512 + k * 256 + hh * 128 : b * 512
                                        + k * 256
                                        + hh * 128
                                        + 128,
                                    ],
                                    rhs=dw_sb[:, b * 128 : b * 128 + 128],
                                    start=(b == 0),
                                    stop=(b == 1),
                                )
                    zss[j] = None
                    y3 = wpool.tile([128, 512], dt.bfloat16, tag="y3", bufs=4)
                    nc.vector.tensor_copy(out=y3[:], in_=p3t[:])
                    y3s[j] = y3[:]

                if 3 <= i < NP + 3:
                    # ---- Ph4: y (h3, (pr, k, w3)); lhsT = Dh const halves ----
                    j = i - 3
                    qd = j // 2
                    if j % 2 == 0:
                        p4ts[qd] = ps4.tile(
                            [128, 512], dt.float32, tag="p4", bufs=1, name=f"p4t{qd}"
                        )
                    p4_r = p4ts[qd][:].rearrange("p (r k n) -> p r k n", r=2, k=2)
                    y3_r = y3s[j].rearrange("p (k hh n) -> p k hh n", k=2, hh=2)
                    for hh in range(2):
                        nc.tensor.matmul(
                            p4_r[:, j % 2],
                            lhsT=dh_sb[:, hh * 128 : hh * 128 + 128],
                            rhs=y3_r[:, :, hh, :],
                            start=(hh == 0),
                            stop=(hh == 1),
                        )
                    y3s[j] = None
                    if j % 2 == 1:
                        # quad output: PSUM -> SBUF (ACT) -> DRAM
                        yq = wpool.tile([128, 512], dt.float32, tag="yq", bufs=3)
                        nc.scalar.activation(yq[:], p4ts[qd][:], AFT.Copy)
                        qt, off = (4 * qd) // CQ, ((4 * qd) % CQ) * H
                        nc.sync.dma_start(
                            out=youts[qt][:, off : off + 512], in_=yq[:]
                        )

    nc.compile()
    _BUILD_CACHE["nc"] = nc
    return nc


def _input_maps(x):
    U = _resize_mat(H, H2)   # (256, 128) upsample
    D = _resize_mat(H2, H)   # (128, 256) antialiased downsample
    try:
        from ml_dtypes import bfloat16
    except ImportError:
        import jax.numpy as jnp  # fallback
        bfloat16 = jnp.bfloat16

    wh_np = np.ascontiguousarray(U.T).astype(bfloat16)     # (h, h2)
    # dh[b, h2local, h3] = D[h3, b*128 + h2local]
    dh_np = np.ascontiguousarray(D.T.reshape(2, 128, 128)).astype(bfloat16)

    in_maps = []
    for i in range(x.shape[0]):
        xr = x[i].reshape(H, W, C).astype(bfloat16)
        m = {"wh": wh_np, "dh": dh_np}
        for q in range(16):
            m[f"xin{q}"] = np.ascontiguousarray(
                xr[:, :, q * 8 : (q + 1) * 8].transpose(0, 2, 1)
            ).reshape(H, W * 8)
        in_maps.append(m)
    return in_maps


def _unshard(results):
    outs = []
    for r in results:
        qs = [np.asarray(r[f"yout{q}"]).reshape(H, 8, W).transpose(0, 2, 1) for q in range(16)]
        outs.append(np.concatenate(qs, axis=2))     # (h3, w3, c)
    return np.stack(outs, axis=0).astype(np.float32)


def run(x, trace=False):
    """Run on 8 NeuronCores. Returns (y, exec_time_ns or None)."""
    from concourse.bass_utils import run_bass_kernel_spmd

    nc = _build_module()
    in_maps = _input_maps(np.asarray(x, dtype=np.float32))
    core_ids = list(range(len(in_maps)))
    res = run_bass_kernel_spmd(nc, in_maps, core_ids, trace=trace)
    return _unshard(res.results), res.exec_time_ns


def kernel(x):
    y, _ = run(x, trace=False)
    return y


def _run_sim(x_batch):
    """CoreSim single-core numerical check (x_batch: (128,128,128) f32)."""
    import concourse.bass_interp as bass_interp

    nc = _build_module()
    sim = bass_interp.CoreSim(nc, trace=False)
    im = _input_maps(x_batch[None])[0]
    for k, v in im.items():
        sim.tensor(k)[:] = v
    sim.simulate()
    qs = [np.asarray(sim.tensor(f"yout{q}")).reshape(H, 8, W).transpose(0, 2, 1) for q in range(16)]
    return np.concatenate(qs, axis=2)
